# revision 16
# baseline (speedup 1.0000x reference)
"""AttentionBlock (GroupNorm32 + 8-head global self-attention + proj + residual)
on 8 TRN2 NeuronCores, data-parallel over batch (B=8 -> 1 image per core).

Per-core layout ([C=512, N=1024] slice, channels on partitions):
  Startup: x shipped twice (bf16 for GN/QKV critical path, f32 late for the
  residual); per-ct DMAs across sync+scalar queues; weights per-kt on gpsimd.
  GroupNorm per-ct as tiles arrive (bn_stats -> per-group selector matmul ->
  expand, affine folded into A,B).  GN psum lives in psSc's upper half; qk
  tiles accumulate nt-outer so those writes are FIFO-gated behind hn3
  (PSUM bank collisions between engines are fatal and the tracker is
  address-level, so bank sharing is managed by emission order).
  Attention: S^T per (m-tile, head-parity) in double-buffered psS halves
  [128,1024]; exp split between ACT (Exp activation) and DVE (Schraudolph
  bit-trick: round(A*s+B) as int16 == bf16 exp approximation, one
  tensor_scalar).  PV per (head, nt) chain accumulates [66,512] with a
  ones-row denominator; evacuation fused with softmax normalization
  (tensor_tensor multiply by DRAM-bounce-broadcast reciprocal); reciprocal
  on repacked [128,2,4] layout.  proj: partial kt accumulation early in
  psS/psV, residual + bias via scalar_tensor_tensor, split output DMA.
"""
import math

import numpy as np

C = 512
NH = 8
D = 64
N = 1024
GROUPS = 32
GS = C // GROUPS  # 16 channels per group
EPS = 1e-5
B = 8
NT = N // 512     # 2 n-tiles of 512
CT = C // 128     # 4 channel tiles
MT = N // 128     # 8 m-tiles (sequence on partitions)

TRACE = False     # test.py flips this for profiling runs

_cache = {}


def _build(with_bias):
    import concourse.bass as bass
    import concourse.bacc as bacc
    import concourse.tile as tile
    import concourse.mybir as mybir

    F32 = mybir.dt.float32
    F32R = mybir.dt.float32r
    BF16 = mybir.dt.bfloat16
    I16 = mybir.dt.int16
    AF = mybir.ActivationFunctionType
    ALU = mybir.AluOpType
    nc = bacc.Bacc("TRN2", target_bir_lowering=False, debug=False,
                   enable_asserts=False, num_devices=1)

    x_d = nc.dram_tensor("x", [C, N], F32, kind="ExternalInput").ap()
    xbf_d = nc.dram_tensor("x_bf", [C, N], BF16, kind="ExternalInput").ap()
    qkv_wT_d = nc.dram_tensor("qkv_wT", [C, 3 * C], BF16, kind="ExternalInput").ap()
    proj_wT_d = nc.dram_tensor("proj_wT", [C, C], BF16, kind="ExternalInput").ap()
    qk_bias_d = nc.dram_tensor("qk_bias", [2 * C, 1], F32, kind="ExternalInput").ap()
    gn_w_d = nc.dram_tensor("gn_w", [C, 1], F32, kind="ExternalInput").ap()
    gn_b_d = nc.dram_tensor("gn_b", [C, 1], F32, kind="ExternalInput").ap()
    proj_be_d = nc.dram_tensor("proj_be", [C, 1], F32, kind="ExternalInput").ap()
    sel_d = nc.dram_tensor("sel", [128, 8], F32R, kind="ExternalInput").ap()
    expander_d = nc.dram_tensor("expander", [8, 128], F32R, kind="ExternalInput").ap()
    rs_dram = nc.dram_tensor("rs_scratch", [NH, N], F32, kind="Internal").ap()
    rs2_dram = nc.dram_tensor("rs2_scratch", [NH, N], F32, kind="Internal").ap()
    out_d = nc.dram_tensor("out", [C, N], F32, kind="ExternalOutput").ap()

    x_r = x_d.rearrange("(t p) n -> p t n", p=128)
    xbf_r = xbf_d.rearrange("(t p) n -> p t n", p=128)
    qkv_r = qkv_wT_d.rearrange("(t p) o -> p t o", p=128)
    proj_r = proj_wT_d.rearrange("(t p) o -> p t o", p=128)
    out_r = out_d.rearrange("(t p) n -> p t n", p=128)

    scale = float(D) ** -0.5
    # Schraudolph exp-as-bf16: bf16_bits(exp(scale*s)) ~= round(A*s + B)
    SCH_A = (2.0 ** 23) / math.log(2.0) / 65536.0 * scale
    SCH_B = (127.0 * 2 ** 23 - 368000.0) / 65536.0
    # which S^T groups (g = 2*mt + hh) run on DVE instead of ACT, pairs 1-3
    EXP_DVE = {1, 4, 7, 10, 13}

    with tile.TileContext(nc) as tc:
        with tc.tile_pool(name="const", bufs=1) as const, \
             tc.tile_pool(name="big", bufs=1) as big, \
             tc.tile_pool(name="pT_pool", bufs=4) as pT_pool, \
             tc.tile_pool(name="small", bufs=2) as small, \
             tc.tile_pool(name="norm", bufs=4) as norm, \
             tc.tile_pool(name="psSa_p", bufs=1, space="PSUM") as psSa_p, \
             tc.tile_pool(name="psSb_p", bufs=1, space="PSUM") as psSb_p, \
             tc.tile_pool(name="psSc_p", bufs=1, space="PSUM") as psSc_p, \
             tc.tile_pool(name="psVa_p", bufs=1, space="PSUM") as psVa_p, \
             tc.tile_pool(name="psVb_p", bufs=1, space="PSUM") as psVb_p:

            # ---- PSUM: 5 fixed tiles (8 banks). Separate tiles because the
            # dependency tracker serializes at tile granularity; S^T/exp uses
            # a 3-buffer rotation so the engine throughput (not the serial
            # st->exp chain) bounds the cadence.
            psSa = psSa_p.tile([128, 1024], F32)  # S^T rot 0, q0, proj 0/3
            psSb = psSb_p.tile([128, 1024], F32)  # S^T rot 1, k0, proj 1
            psSc = psSc_p.tile([128, 1024], F32)  # S^T rot 2, GN, qk rest, proj 2
            psVa = psVa_p.tile([128, 512], F32)   # v even, PV chains 0,2
            psVb = psVb_p.tile([128, 512], F32)   # v odd, PV chains 1,3

            # ---- constants / weights (gpsimd queue) ----
            sel = const.tile([128, 8], F32R)
            expander = const.tile([8, 128], F32R)
            gn_w = const.tile([128, CT, 1], F32)
            gn_b = const.tile([128, CT, 1], F32)
            proj_be = const.tile([128, CT, 1], F32)
            qkv_wT = const.tile([128, CT, 3 * C], BF16)
            proj_wT = const.tile([128, CT, C], BF16)
            eps_t = const.tile([8, 1], F32)
            qk_bias = const.tile([128, 2 * CT, 1], F32)

            nc.gpsimd.dma_start(out=sel, in_=sel_d)
            nc.gpsimd.dma_start(out=expander, in_=expander_d)
            nc.gpsimd.dma_start(out=gn_w, in_=gn_w_d.rearrange("(t p) o -> p t o", p=128))
            nc.gpsimd.dma_start(out=gn_b, in_=gn_b_d.rearrange("(t p) o -> p t o", p=128))
            nc.gpsimd.dma_start(out=proj_be, in_=proj_be_d.rearrange("(t p) o -> p t o", p=128))
            if with_bias:
                nc.gpsimd.dma_start(out=qk_bias,
                                    in_=qk_bias_d.rearrange("(t p) o -> p t o", p=128))
            for kt in range(CT):
                nc.gpsimd.dma_start(out=qkv_wT[:, kt, :], in_=qkv_r[:, kt, :])
            nc.gpsimd.dma_start(out=proj_wT, in_=proj_r)
            nc.vector.memset(eps_t, EPS)

            # ---- input x (bf16, critical path): 4 per-ct DMAs, 2 queues ----
            xb_sb = big.tile([128, CT, N], BF16)
            for ci in range(CT):
                q = nc.sync if ci % 2 == 0 else nc.scalar
                q.dma_start(out=xb_sb[:, ci, :], in_=xbf_r[:, ci, :])
            # f32 x for the residual, loaded behind the weights (gpsimd)
            x_sb = big.tile([128, CT, N], F32)
            nc.gpsimd.dma_start(out=x_sb, in_=x_r)

            # ---- GroupNorm, per-ct pipeline (psum in psX bank 1) ----
            hn = big.tile([128, CT, N], BF16)
            for ci in range(CT):
                bstats = norm.tile([128, 2, 6], F32, tag="bst")
                xv = xb_sb[:, ci, :].rearrange("p (s n) -> p s n", s=2)
                for s in range(2):
                    nc.vector.bn_stats(out=bstats[:, s, :], in_=xv[:, s, :])
                mv = norm.tile([128, 2], F32, tag="mv")
                nc.vector.bn_aggr(out=mv, in_=bstats)
                # srhs: col0 = mean_c, col1 = var_c + mean_c^2 (= E[x^2])
                srhs = norm.tile([128, 2], F32R, tag="srhs")
                nc.vector.tensor_copy(out=srhs[:, 0:1], in_=mv[:, 0:1])
                nc.vector.tensor_tensor(out=srhs[:, 1:2], in0=mv[:, 0:1],
                                        in1=mv[:, 0:1], op=ALU.mult)
                nc.vector.tensor_tensor(out=srhs[:, 1:2], in0=srhs[:, 1:2],
                                        in1=mv[:, 1:2], op=ALU.add)
                gp = psSc[0:8, 512 + 16 * ci:512 + 16 * ci + 2]
                nc.tensor.matmul(gp, sel[:], srhs[:], start=True, stop=True)
                gms = norm.tile([8, 2], F32, tag="gms")
                nc.vector.tensor_copy(out=gms, in_=gp)
                gvar = norm.tile([8, 1], F32, tag="gvar")
                grp2 = norm.tile([8, 2], F32R, tag="grp2")
                nc.vector.tensor_tensor(out=gvar, in0=gms[:, 0:1], in1=gms[:, 0:1],
                                        op=ALU.mult)
                nc.vector.tensor_tensor(out=gvar, in0=gms[:, 1:2], in1=gvar,
                                        op=ALU.subtract)
                nc.scalar.activation(out=gvar, in_=gvar, func=AF.Sqrt, bias=eps_t,
                                     scale=1.0)
                nc.vector.reciprocal(out=gvar, in_=gvar)
                nc.vector.tensor_copy(out=grp2[:, 0:1], in_=gms[:, 0:1])
                nc.vector.tensor_copy(out=grp2[:, 1:2], in_=gvar)
                ep = psSc[:, 576 + 16 * ci:576 + 16 * ci + 2]
                nc.tensor.matmul(ep, expander[:], grp2[:], start=True, stop=True)
                A = norm.tile([128, 1], F32, tag="A")
                Bb = norm.tile([128, 1], F32, tag="Bb")
                nc.vector.tensor_tensor(out=A, in0=ep[:, 1:2], in1=gn_w[:, ci, :],
                                        op=ALU.mult)
                nc.vector.tensor_tensor(out=Bb, in0=ep[:, 0:1], in1=A, op=ALU.mult)
                nc.vector.tensor_tensor(out=Bb, in0=gn_b[:, ci, :], in1=Bb,
                                        op=ALU.subtract)
                nc.vector.tensor_scalar(out=hn[:, ci, :], in0=xb_sb[:, ci, :],
                                        scalar1=A, scalar2=Bb,
                                        op0=ALU.mult, op1=ALU.add)

            # ---- data tiles for attention ----
            q_sb = big.tile([128, CT, N], BF16)
            k_sb = big.tile([128, CT, N], BF16)
            vT = big.tile([128, MT, NH, D + 2], BF16)
            oT = big.tile([128, CT, N], BF16)
            out_sb = big.tile([128, CT, N], F32)
            nc.vector.memset(vT[:, :, :, D:D + 1], 1.0)
            nc.vector.memset(vT[:, :, :, D + 1:D + 2], 0.0)

            # ---- emission helpers ----
            def qk_tile(i, on_act, bases):
                """QKV tile i (0-3 = q ct, 4-7 = k ct). nt-outer: the nt1
                chain's first MM is FIFO-gated behind nt0's kt3 (needs hn3),
                so psSc's GN half is only written once GN is done."""
                dest = q_sb if i < CT else k_sb
                ci = i % CT
                base = bases[0]
                for nt in range(NT):
                    for kt in range(CT):
                        nc.tensor.matmul(
                            base[:, 512 * nt:512 * (nt + 1)],
                            qkv_wT[:, kt, 128 * i:128 * (i + 1)],
                            hn[:, kt, 512 * nt:512 * (nt + 1)],
                            start=(kt == 0), stop=(kt == CT - 1))
                if with_bias:
                    nc.vector.tensor_scalar(out=dest[:, ci, :],
                                            in0=base[:, 0:1024],
                                            scalar1=qk_bias[:, i, :],
                                            scalar2=None, op0=ALU.add)
                elif on_act:
                    nc.scalar.activation(out=dest[:, ci, :],
                                         in_=base[:, 0:1024], func=AF.Copy)
                else:
                    nc.vector.tensor_copy(out=dest[:, ci, :],
                                          in_=base[:, 0:1024])

            def v_tile(mt):
                """v for n-block mt, evac to vT (head-interleaved)."""
                base = psVa if mt % 2 == 0 else psVb
                for kt in range(CT):
                    nc.tensor.matmul(base[:, 0:512],
                                     hn[:, kt, 128 * mt:128 * (mt + 1)],
                                     qkv_wT[:, kt, 2 * C:3 * C],
                                     start=(kt == 0), stop=(kt == CT - 1))
                nc.vector.tensor_copy(
                    out=vT[:, mt, :, 0:D],
                    in_=base[:, 0:512].rearrange("p (h d) -> p h d", h=NH))

            pT_tiles = {}

            def st_half(t, g, on_dve, bufs3):
                """S^T for head pair t, group g = 2*mt + hh, into a rotating
                psS buffer; exp on ACT or DVE-Schraudolph."""
                mt, hh = g // 2, g % 2
                qp = hh * 64
                half = bufs3[g % len(bufs3)]
                for nt in range(NT):
                    nc.tensor.matmul(
                        half[:, 512 * nt:512 * (nt + 1)],
                        k_sb[qp:qp + 64, t, 128 * mt:128 * (mt + 1)],
                        q_sb[qp:qp + 64, t, 512 * nt:512 * (nt + 1)],
                        start=True, stop=True)
                pTt = pT_tiles[t]
                if on_dve:
                    nc.vector.tensor_scalar(
                        out=pTt.bitcast(I16)[:, hh, mt, :], in0=half,
                        scalar1=SCH_A, scalar2=SCH_B,
                        op0=ALU.mult, op1=ALU.add)
                else:
                    nc.scalar.activation(out=pTt[:, hh, mt, :], in_=half,
                                         func=AF.Exp, scale=scale)

            # PV chain psum slots: chains 0,2 share psVa; 1,3 share psVb
            # (sequential in time: chain c evacs before chain c+2 starts)
            def pv_slot(chain):
                return (psVa if chain % 2 == 0 else psVb)[0:D + 2, 0:512]

            def pv_chunk(t, chain, part):
                """4 MMs of PV chain (0=h_ev/nt0, 1=h_od/nt0, 2=h_ev/nt1,
                3=h_od/nt1), part 0/1 = m-tiles 0-3 / 4-7."""
                hh = chain % 2
                nt = chain // 2
                h = 2 * t + hh
                slot = pv_slot(chain)
                pTt = pT_tiles[t]
                for mt in range(4 * part, 4 * part + 4):
                    nc.tensor.matmul(slot,
                                     vT[:, mt, h, :],
                                     pTt[:, hh, mt, 512 * nt:512 * (nt + 1)],
                                     start=(mt == 0), stop=(mt == MT - 1))
                if part == 1:
                    rs = small.tile([1, 512], F32, tag="rs", bufs=4,
                                    name=f"rs_{h}_{nt}")
                    nc.vector.tensor_copy(out=rs, in_=slot[D:D + 1, :])
                    nc.sync.dma_start(out=rs_dram[h:h + 1, 512 * nt:512 * (nt + 1)],
                                      in_=rs)
                    recip_chain(t, chain)
                    bc_load(t, chain)

            def recip_chain(t, chain):
                """1/denominator for one (head, nt) chain, repacked [128,1,4]
                so the iterative divide runs at free-dim 4, not 512."""
                hh, nt = chain % 2, chain // 2
                h = 2 * t + hh
                rg = small.tile([128, 1, 4], F32, tag="rg", bufs=4,
                                name=f"rg_{t}_{chain}")
                src = rs_dram[h:h + 1, 512 * nt:512 * (nt + 1)]
                nc.scalar.dma_start(out=rg,
                                    in_=src.rearrange("h (p f) -> p h f", p=128))
                nc.vector.reciprocal(out=rg, in_=rg)
                nc.scalar.dma_start(
                    out=rs2_dram[h:h + 1, 512 * nt:512 * (nt + 1)].rearrange(
                        "h (p f) -> p h f", p=128),
                    in_=rg)

            bc_tiles = {}

            def bc_load(t, chain):
                """Broadcast 1/denom to 64 partitions via DRAM-bounce DMA."""
                hh, nt = chain % 2, chain // 2
                h = 2 * t + hh
                qp = hh * 64
                key = (t, nt)
                if key not in bc_tiles:
                    bc_tiles[key] = small.tile([128, 512], F32, tag=f"bc{nt}",
                                               bufs=2, name=f"bc_{t}_{nt}")
                bc = bc_tiles[key]
                srcap = rs2_dram[h:h + 1, 512 * nt:512 * (nt + 1)]
                nc.gpsimd.dma_start(out=bc[qp:qp + 64, :],
                                    in_=bass.AP(tensor=srcap.tensor,
                                                offset=srcap.offset,
                                                ap=[[0, 64]] + list(srcap.ap[1:])))

            def pv_evac(t, chain):
                """Fused evacuate+normalize: oT = psum_o * (1/denom)."""
                hh, nt = chain % 2, chain // 2
                qp = hh * 64
                slot = pv_slot(chain)
                bc = bc_tiles[(t, nt)]
                nc.vector.tensor_tensor(
                    out=oT[qp:qp + 64, t, 512 * nt:512 * (nt + 1)],
                    in0=slot[0:D, :], in1=bc[qp:qp + 64, :], op=ALU.mult)

            def proj_chunk(ot, kts, base, first, last):
                for kt in kts:
                    for nt in range(NT):
                        nc.tensor.matmul(
                            base[:, 512 * nt:512 * (nt + 1)],
                            proj_wT[:, kt, 128 * ot:128 * (ot + 1)],
                            oT[:, kt, 512 * nt:512 * (nt + 1)],
                            start=(kt == kts[0] and first),
                            stop=(kt == kts[-1] and last))

            def proj_finish(ot, base):
                nc.vector.scalar_tensor_tensor(
                    out=out_sb[:, ot, :], in0=base[:, 0:1024],
                    scalar=proj_be[:, ot, :], in1=x_sb[:, ot, :],
                    op0=ALU.add, op1=ALU.add)
                q = nc.sync if ot % 2 == 0 else nc.scalar
                q.dma_start(out=out_r[:, ot, :], in_=out_sb[:, ot, :])

            def alloc_pT(t):
                pT_tiles[t] = pT_pool.tile([128, 2, MT, N], BF16, tag="pT", bufs=2,
                                           name=f"pT_{t}")

            # ---- pipeline emission ----
            # q0, k0 first (psSa/psSb) so pair-0 S^T can start early
            qk_tile(0, on_act=True, bases=[psSa])
            qk_tile(4, on_act=True, bases=[psSb])

            # pair 0: S^T+exp (all ACT; PE-bound anyway) + v tiles (psV) +
            # remaining qk tiles (psX)
            alloc_pT(0)
            rest = [1, 5, 2, 6, 3, 7]        # q1,k1,q2,k2,q3,k3
            for g in range(16):
                if g % 2 == 0:
                    v_tile(g // 2)
                elif g < 13:
                    qk_tile(rest[g // 2], on_act=False, bases=[psSc])
                st_half(0, g, on_dve=False, bufs3=[psSa, psSb])

            # pairs 1..3: PV(t-1) + S^T(t) + exp (ACT/DVE split) + stage_b
            for t in range(1, 4):
                alloc_pT(t)
                pv = t - 1
                for g in range(16):
                    if g == 2 and pv >= 1:
                        pv_evac(pv - 1, 3)
                    if g % 2 == 0:
                        pv_chunk(pv, g // 4, (g // 2) % 2)
                    st_half(t, g, on_dve=(g in EXP_DVE),
                            bufs3=[psSa, psSb, psSc])
                    if g in (6, 10, 14):
                        pv_evac(pv, (g - 6) // 4)
                del pT_tiles[t - 1]

            # PV(3) + early proj partials (ot0/ot1/ot2 in psSa/psSb/psSc)
            proj_sched = {1: (0, 0), 2: (1, 0), 3: (0, 1), 4: (1, 1),
                          5: (0, 2), 7: (1, 2), 9: (2, 0), 11: (2, 1),
                          13: (2, 2)}
            proj_base = {0: psSa, 1: psSb, 2: psSc}
            for g in range(16):
                if g == 2:
                    pv_evac(2, 3)
                ps = proj_sched.get(g)
                if ps is not None:
                    ot, kt = ps
                    proj_chunk(ot, [kt], proj_base[ot],
                               first=(kt == 0), last=False)
                if g % 2 == 0:
                    pv_chunk(3, g // 4, (g // 2) % 2)
                if g in (6, 10, 14):
                    pv_evac(3, (g - 6) // 4)
            pv_evac(3, 3)

            # ---- finish projection + residual ----
            proj_chunk(0, [3], psSa, first=False, last=True)
            proj_finish(0, psSa)
            proj_chunk(1, [3], psSb, first=False, last=True)
            proj_finish(1, psSb)
            proj_chunk(2, [3], psSc, first=False, last=True)
            proj_finish(2, psSc)
            proj_chunk(3, [0, 1, 2, 3], psSa, first=True, last=True)
            proj_finish(3, psSa)

    nc.compile()
    return nc


def _host_prep(x, gn_w, gn_b, qkv_w, qkv_b, proj_w, proj_b):
    xf = np.ascontiguousarray(x.reshape(B, C, N), dtype=np.float32)
    import ml_dtypes
    qkv_wT = np.ascontiguousarray(qkv_w.T).astype(ml_dtypes.bfloat16)
    proj_wT = np.ascontiguousarray(proj_w.T).astype(ml_dtypes.bfloat16)
    proj_be = (proj_b + proj_w @ qkv_b[2 * C:]).astype(np.float32).reshape(C, 1)
    qk_bias = np.ascontiguousarray(qkv_b[:2 * C], dtype=np.float32).reshape(2 * C, 1)
    cid = np.arange(128)
    sel = ((cid[:, None] // GS == np.arange(8)[None, :]) / GS).astype(np.float32)
    expander = np.ascontiguousarray(
        (cid[:, None] // GS == np.arange(8)[None, :]).T.astype(np.float32))
    shared = {
        "qkv_wT": qkv_wT, "proj_wT": proj_wT, "qk_bias": qk_bias,
        "gn_w": np.asarray(gn_w, np.float32).reshape(C, 1),
        "gn_b": np.asarray(gn_b, np.float32).reshape(C, 1),
        "proj_be": proj_be, "sel": sel, "expander": expander,
    }
    return [{**shared, "x": np.ascontiguousarray(xf[i]),
             "x_bf": xf[i].astype(ml_dtypes.bfloat16)} for i in range(B)]


def kernel(x, gn_w, gn_b, qkv_w, qkv_b, proj_w, proj_b):
    from concourse import bass_utils
    in_maps = _host_prep(np.asarray(x), np.asarray(gn_w), np.asarray(gn_b),
                         np.asarray(qkv_w), np.asarray(qkv_b),
                         np.asarray(proj_w), np.asarray(proj_b))
    with_bias = bool(np.any(np.asarray(qkv_b)[:2 * C] != 0.0))
    key = ("nc", with_bias)
    if key not in _cache:
        _cache[key] = _build(with_bias)
    res = bass_utils.run_bass_kernel_spmd(_cache[key], in_maps,
                                          core_ids=list(range(B)), trace=TRACE)
    _cache["last_result"] = res
    out = np.stack([res.results[i]["out"] for i in range(B)])
    return out.reshape(B, C, 32, 32).astype(np.float32)


# revision 17
# speedup vs baseline: 1.0054x; 1.0054x over previous
"""AttentionBlock (GroupNorm32 + 8-head global self-attention + proj + residual)
on 8 TRN2 NeuronCores, data-parallel over batch (B=8 -> 1 image per core).

Per-core layout ([C=512, N=1024] slice, channels on partitions):
  Startup: x shipped twice (bf16 for GN/QKV critical path, f32 late for the
  residual); per-ct DMAs across sync+scalar queues; weights per-kt on gpsimd.
  GroupNorm per-ct as tiles arrive (bn_stats -> per-group selector matmul ->
  expand, affine folded into A,B).  GN psum lives in psSc's upper half; qk
  tiles accumulate nt-outer so those writes are FIFO-gated behind hn3
  (PSUM bank collisions between engines are fatal and the tracker is
  address-level, so bank sharing is managed by emission order).
  Attention: S^T per (m-tile, head-parity) in double-buffered psS halves
  [128,1024]; exp split between ACT (Exp activation) and DVE (Schraudolph
  bit-trick: round(A*s+B) as int16 == bf16 exp approximation, one
  tensor_scalar).  PV per (head, nt) chain accumulates [66,512] with a
  ones-row denominator; evacuation fused with softmax normalization
  (tensor_tensor multiply by DRAM-bounce-broadcast reciprocal); reciprocal
  on repacked [128,2,4] layout.  proj: partial kt accumulation early in
  psS/psV, residual + bias via scalar_tensor_tensor, split output DMA.
"""
import math

import numpy as np

C = 512
NH = 8
D = 64
N = 1024
GROUPS = 32
GS = C // GROUPS  # 16 channels per group
EPS = 1e-5
B = 8
NT = N // 512     # 2 n-tiles of 512
CT = C // 128     # 4 channel tiles
MT = N // 128     # 8 m-tiles (sequence on partitions)

TRACE = False     # test.py flips this for profiling runs

_cache = {}


def _build(with_bias):
    import concourse.bass as bass
    import concourse.bacc as bacc
    import concourse.tile as tile
    import concourse.mybir as mybir

    F32 = mybir.dt.float32
    F32R = mybir.dt.float32r
    BF16 = mybir.dt.bfloat16
    I16 = mybir.dt.int16
    AF = mybir.ActivationFunctionType
    ALU = mybir.AluOpType
    nc = bacc.Bacc("TRN2", target_bir_lowering=False, debug=False,
                   enable_asserts=False, num_devices=1)

    x_d = nc.dram_tensor("x", [C, N], F32, kind="ExternalInput").ap()
    xbf_d = nc.dram_tensor("x_bf", [C, N], BF16, kind="ExternalInput").ap()
    qkv_wT_d = nc.dram_tensor("qkv_wT", [C, 3 * C], BF16, kind="ExternalInput").ap()
    proj_wT_d = nc.dram_tensor("proj_wT", [C, C], BF16, kind="ExternalInput").ap()
    qk_bias_d = nc.dram_tensor("qk_bias", [2 * C, 1], F32, kind="ExternalInput").ap()
    gn_w_d = nc.dram_tensor("gn_w", [C, 1], F32, kind="ExternalInput").ap()
    gn_b_d = nc.dram_tensor("gn_b", [C, 1], F32, kind="ExternalInput").ap()
    proj_be_d = nc.dram_tensor("proj_be", [C, 1], F32, kind="ExternalInput").ap()
    sel_d = nc.dram_tensor("sel", [128, 8], F32R, kind="ExternalInput").ap()
    expander_d = nc.dram_tensor("expander", [8, 128], F32R, kind="ExternalInput").ap()
    rs_dram = nc.dram_tensor("rs_scratch", [NH, N], F32, kind="Internal").ap()
    rs2_dram = nc.dram_tensor("rs2_scratch", [NH, N], F32, kind="Internal").ap()
    out_d = nc.dram_tensor("out", [C, N], F32, kind="ExternalOutput").ap()

    x_r = x_d.rearrange("(t p) n -> p t n", p=128)
    xbf_r = xbf_d.rearrange("(t p) n -> p t n", p=128)
    qkv_r = qkv_wT_d.rearrange("(t p) o -> p t o", p=128)
    proj_r = proj_wT_d.rearrange("(t p) o -> p t o", p=128)
    out_r = out_d.rearrange("(t p) n -> p t n", p=128)

    scale = float(D) ** -0.5
    # Schraudolph exp-as-bf16: bf16_bits(exp(scale*s)) ~= round(A*s + B)
    SCH_A = (2.0 ** 23) / math.log(2.0) / 65536.0 * scale
    SCH_B = (127.0 * 2 ** 23 - 368000.0) / 65536.0
    # which S^T groups (g = 2*mt + hh) run on DVE instead of ACT, pairs 1-3
    EXP_DVE = {1, 4, 7, 10, 13}

    with tile.TileContext(nc) as tc:
        with tc.tile_pool(name="const", bufs=1) as const, \
             tc.tile_pool(name="big", bufs=1) as big, \
             tc.tile_pool(name="pT_pool", bufs=4) as pT_pool, \
             tc.tile_pool(name="small", bufs=2) as small, \
             tc.tile_pool(name="norm", bufs=4) as norm, \
             tc.tile_pool(name="psSa_p", bufs=1, space="PSUM") as psSa_p, \
             tc.tile_pool(name="psSb_p", bufs=1, space="PSUM") as psSb_p, \
             tc.tile_pool(name="psSc_p", bufs=1, space="PSUM") as psSc_p, \
             tc.tile_pool(name="psVa_p", bufs=1, space="PSUM") as psVa_p, \
             tc.tile_pool(name="psVb_p", bufs=1, space="PSUM") as psVb_p:

            # ---- PSUM: 5 fixed tiles (8 banks). Separate tiles because the
            # dependency tracker serializes at tile granularity; S^T/exp uses
            # a 3-buffer rotation so the engine throughput (not the serial
            # st->exp chain) bounds the cadence.
            psSa = psSa_p.tile([128, 1024], F32)  # S^T rot 0, q0, proj 0/3
            psSb = psSb_p.tile([128, 1024], F32)  # S^T rot 1, k0, proj 1
            psSc = psSc_p.tile([128, 1024], F32)  # S^T rot 2, GN, qk rest, proj 2
            psVa = psVa_p.tile([128, 512], F32)   # v even, PV chains 0,2
            psVb = psVb_p.tile([128, 512], F32)   # v odd, PV chains 1,3

            # ---- constants / weights (gpsimd queue) ----
            sel = const.tile([128, 8], F32R)
            expander = const.tile([8, 128], F32R)
            gn_w = const.tile([128, CT, 1], F32)
            gn_b = const.tile([128, CT, 1], F32)
            proj_be = const.tile([128, CT, 1], F32)
            qkv_wT = const.tile([128, CT, 3 * C], BF16)
            proj_wT = const.tile([128, CT, C], BF16)
            eps_t = const.tile([8, 1], F32)
            qk_bias = const.tile([128, 2 * CT, 1], F32)

            nc.gpsimd.dma_start(out=sel, in_=sel_d)
            nc.gpsimd.dma_start(out=expander, in_=expander_d)
            nc.gpsimd.dma_start(out=gn_w, in_=gn_w_d.rearrange("(t p) o -> p t o", p=128))
            nc.gpsimd.dma_start(out=gn_b, in_=gn_b_d.rearrange("(t p) o -> p t o", p=128))
            nc.gpsimd.dma_start(out=proj_be, in_=proj_be_d.rearrange("(t p) o -> p t o", p=128))
            if with_bias:
                nc.gpsimd.dma_start(out=qk_bias,
                                    in_=qk_bias_d.rearrange("(t p) o -> p t o", p=128))
            for kt in range(CT):
                nc.gpsimd.dma_start(out=qkv_wT[:, kt, :], in_=qkv_r[:, kt, :])
            nc.gpsimd.dma_start(out=proj_wT, in_=proj_r)
            nc.vector.memset(eps_t, EPS)

            # ---- input x (bf16, critical path): 4 per-ct DMAs, 2 queues ----
            xb_sb = big.tile([128, CT, N], BF16)
            for ci in range(CT):
                q = nc.sync if ci % 2 == 0 else nc.scalar
                q.dma_start(out=xb_sb[:, ci, :], in_=xbf_r[:, ci, :])
            # f32 x for the residual, loaded behind the weights (gpsimd)
            x_sb = big.tile([128, CT, N], F32)
            nc.gpsimd.dma_start(out=x_sb, in_=x_r)

            # ---- GroupNorm, per-ct pipeline (psum in psX bank 1) ----
            hn = big.tile([128, CT, N], BF16)
            for ci in range(CT):
                bstats = norm.tile([128, 2, 6], F32, tag="bst")
                xv = xb_sb[:, ci, :].rearrange("p (s n) -> p s n", s=2)
                for s in range(2):
                    nc.vector.bn_stats(out=bstats[:, s, :], in_=xv[:, s, :])
                mv = norm.tile([128, 2], F32, tag="mv")
                nc.vector.bn_aggr(out=mv, in_=bstats)
                # srhs: col0 = mean_c, col1 = var_c + mean_c^2 (= E[x^2])
                srhs = norm.tile([128, 2], F32R, tag="srhs")
                nc.vector.tensor_copy(out=srhs[:, 0:1], in_=mv[:, 0:1])
                nc.vector.tensor_tensor(out=srhs[:, 1:2], in0=mv[:, 0:1],
                                        in1=mv[:, 0:1], op=ALU.mult)
                nc.vector.tensor_tensor(out=srhs[:, 1:2], in0=srhs[:, 1:2],
                                        in1=mv[:, 1:2], op=ALU.add)
                gp = psSc[0:8, 512 + 16 * ci:512 + 16 * ci + 2]
                nc.tensor.matmul(gp, sel[:], srhs[:], start=True, stop=True)
                gms = norm.tile([8, 2], F32, tag="gms")
                nc.vector.tensor_copy(out=gms, in_=gp)
                gvar = norm.tile([8, 1], F32, tag="gvar")
                grp2 = norm.tile([8, 2], F32R, tag="grp2")
                nc.vector.tensor_tensor(out=gvar, in0=gms[:, 0:1], in1=gms[:, 0:1],
                                        op=ALU.mult)
                nc.vector.tensor_tensor(out=gvar, in0=gms[:, 1:2], in1=gvar,
                                        op=ALU.subtract)
                nc.scalar.activation(out=gvar, in_=gvar, func=AF.Sqrt, bias=eps_t,
                                     scale=1.0)
                nc.vector.reciprocal(out=gvar, in_=gvar)
                nc.vector.tensor_copy(out=grp2[:, 0:1], in_=gms[:, 0:1])
                nc.vector.tensor_copy(out=grp2[:, 1:2], in_=gvar)
                ep = psSc[:, 576 + 16 * ci:576 + 16 * ci + 2]
                nc.tensor.matmul(ep, expander[:], grp2[:], start=True, stop=True)
                A = norm.tile([128, 1], F32, tag="A")
                Bb = norm.tile([128, 1], F32, tag="Bb")
                nc.vector.tensor_tensor(out=A, in0=ep[:, 1:2], in1=gn_w[:, ci, :],
                                        op=ALU.mult)
                nc.vector.tensor_tensor(out=Bb, in0=ep[:, 0:1], in1=A, op=ALU.mult)
                nc.vector.tensor_tensor(out=Bb, in0=gn_b[:, ci, :], in1=Bb,
                                        op=ALU.subtract)
                nc.vector.tensor_scalar(out=hn[:, ci, :], in0=xb_sb[:, ci, :],
                                        scalar1=A, scalar2=Bb,
                                        op0=ALU.mult, op1=ALU.add)

            # ---- data tiles for attention ----
            q_sb = big.tile([128, CT, N], BF16)
            k_sb = big.tile([128, CT, N], BF16)
            vT = big.tile([128, MT, NH, D + 2], BF16)
            oT = big.tile([128, CT, N], BF16)
            out_sb = big.tile([128, CT, N], F32)
            nc.vector.memset(vT[:, :, :, D:D + 1], 1.0)
            nc.vector.memset(vT[:, :, :, D + 1:D + 2], 0.0)

            # ---- emission helpers ----
            def qk_tile(i, on_act, bases):
                """QKV tile i (0-3 = q ct, 4-7 = k ct). nt-outer: the nt1
                chain's first MM is FIFO-gated behind nt0's kt3 (needs hn3),
                so psSc's GN half is only written once GN is done."""
                dest = q_sb if i < CT else k_sb
                ci = i % CT
                base = bases[0]
                for nt in range(NT):
                    for kt in range(CT):
                        nc.tensor.matmul(
                            base[:, 512 * nt:512 * (nt + 1)],
                            qkv_wT[:, kt, 128 * i:128 * (i + 1)],
                            hn[:, kt, 512 * nt:512 * (nt + 1)],
                            start=(kt == 0), stop=(kt == CT - 1))
                if with_bias:
                    nc.vector.tensor_scalar(out=dest[:, ci, :],
                                            in0=base[:, 0:1024],
                                            scalar1=qk_bias[:, i, :],
                                            scalar2=None, op0=ALU.add)
                elif on_act:
                    nc.scalar.activation(out=dest[:, ci, :],
                                         in_=base[:, 0:1024], func=AF.Copy)
                else:
                    nc.vector.tensor_copy(out=dest[:, ci, :],
                                          in_=base[:, 0:1024])

            def v_tile(mt):
                """v for n-block mt, evac to vT (head-interleaved)."""
                base = psVa if mt % 2 == 0 else psVb
                for kt in range(CT):
                    nc.tensor.matmul(base[:, 0:512],
                                     hn[:, kt, 128 * mt:128 * (mt + 1)],
                                     qkv_wT[:, kt, 2 * C:3 * C],
                                     start=(kt == 0), stop=(kt == CT - 1))
                nc.vector.tensor_copy(
                    out=vT[:, mt, :, 0:D],
                    in_=base[:, 0:512].rearrange("p (h d) -> p h d", h=NH))

            pT_tiles = {}

            def st_half(t, g, on_dve, bufs3):
                """S^T for head pair t, group g = 2*mt + hh, into a rotating
                psS buffer; exp on ACT or DVE-Schraudolph."""
                mt, hh = g // 2, g % 2
                qp = hh * 64
                half = bufs3[g % len(bufs3)]
                for nt in range(NT):
                    nc.tensor.matmul(
                        half[:, 512 * nt:512 * (nt + 1)],
                        k_sb[qp:qp + 64, t, 128 * mt:128 * (mt + 1)],
                        q_sb[qp:qp + 64, t, 512 * nt:512 * (nt + 1)],
                        start=True, stop=True)
                pTt = pT_tiles[t]
                if on_dve:
                    nc.vector.tensor_scalar(
                        out=pTt.bitcast(I16)[:, hh, mt, :], in0=half,
                        scalar1=SCH_A, scalar2=SCH_B,
                        op0=ALU.mult, op1=ALU.add)
                else:
                    nc.scalar.activation(out=pTt[:, hh, mt, :], in_=half,
                                         func=AF.Exp, scale=scale)

            # PV chain psum slots: chains 0,2 share psVa; 1,3 share psVb
            # (sequential in time: chain c evacs before chain c+2 starts)
            def pv_slot(chain):
                return (psVa if chain % 2 == 0 else psVb)[0:D + 2, 0:512]

            def pv_chunk(t, chain, part):
                """4 MMs of PV chain (0=h_ev/nt0, 1=h_od/nt0, 2=h_ev/nt1,
                3=h_od/nt1), part 0/1 = m-tiles 0-3 / 4-7."""
                hh = chain % 2
                nt = chain // 2
                h = 2 * t + hh
                slot = pv_slot(chain)
                pTt = pT_tiles[t]
                for mt in range(4 * part, 4 * part + 4):
                    nc.tensor.matmul(slot,
                                     vT[:, mt, h, :],
                                     pTt[:, hh, mt, 512 * nt:512 * (nt + 1)],
                                     start=(mt == 0), stop=(mt == MT - 1))
                if part == 1:
                    rs = small.tile([1, 512], F32, tag="rs", bufs=4,
                                    name=f"rs_{h}_{nt}")
                    nc.vector.tensor_copy(out=rs, in_=slot[D:D + 1, :])
                    nc.sync.dma_start(out=rs_dram[h:h + 1, 512 * nt:512 * (nt + 1)],
                                      in_=rs)
                    recip_chain(t, chain)
                    bc_load(t, chain)

            def recip_chain(t, chain):
                """1/denominator for one (head, nt) chain, repacked [128,1,4]
                so the iterative divide runs at free-dim 4, not 512."""
                hh, nt = chain % 2, chain // 2
                h = 2 * t + hh
                rg = small.tile([128, 1, 4], F32, tag="rg", bufs=4,
                                name=f"rg_{t}_{chain}")
                src = rs_dram[h:h + 1, 512 * nt:512 * (nt + 1)]
                nc.gpsimd.dma_start(out=rg,
                                    in_=src.rearrange("h (p f) -> p h f", p=128))
                nc.vector.reciprocal(out=rg, in_=rg)
                nc.sync.dma_start(
                    out=rs2_dram[h:h + 1, 512 * nt:512 * (nt + 1)].rearrange(
                        "h (p f) -> p h f", p=128),
                    in_=rg)

            bc_tiles = {}

            def bc_load(t, chain):
                """Broadcast 1/denom to 64 partitions via DRAM-bounce DMA."""
                hh, nt = chain % 2, chain // 2
                h = 2 * t + hh
                qp = hh * 64
                key = (t, nt)
                if key not in bc_tiles:
                    bc_tiles[key] = small.tile([128, 512], F32, tag=f"bc{nt}",
                                               bufs=2, name=f"bc_{t}_{nt}")
                bc = bc_tiles[key]
                srcap = rs2_dram[h:h + 1, 512 * nt:512 * (nt + 1)]
                nc.gpsimd.dma_start(out=bc[qp:qp + 64, :],
                                    in_=bass.AP(tensor=srcap.tensor,
                                                offset=srcap.offset,
                                                ap=[[0, 64]] + list(srcap.ap[1:])))

            def pv_evac(t, chain):
                """Fused evacuate+normalize: oT = psum_o * (1/denom)."""
                hh, nt = chain % 2, chain // 2
                qp = hh * 64
                slot = pv_slot(chain)
                bc = bc_tiles[(t, nt)]
                nc.vector.tensor_tensor(
                    out=oT[qp:qp + 64, t, 512 * nt:512 * (nt + 1)],
                    in0=slot[0:D, :], in1=bc[qp:qp + 64, :], op=ALU.mult)

            def proj_chunk(ot, kts, base, first, last):
                for kt in kts:
                    for nt in range(NT):
                        nc.tensor.matmul(
                            base[:, 512 * nt:512 * (nt + 1)],
                            proj_wT[:, kt, 128 * ot:128 * (ot + 1)],
                            oT[:, kt, 512 * nt:512 * (nt + 1)],
                            start=(kt == kts[0] and first),
                            stop=(kt == kts[-1] and last))

            def proj_finish(ot, base):
                nc.vector.scalar_tensor_tensor(
                    out=out_sb[:, ot, :], in0=base[:, 0:1024],
                    scalar=proj_be[:, ot, :], in1=x_sb[:, ot, :],
                    op0=ALU.add, op1=ALU.add)
                q = nc.sync if ot % 2 == 0 else nc.scalar
                q.dma_start(out=out_r[:, ot, :], in_=out_sb[:, ot, :])

            def alloc_pT(t):
                pT_tiles[t] = pT_pool.tile([128, 2, MT, N], BF16, tag="pT", bufs=2,
                                           name=f"pT_{t}")

            # ---- pipeline emission ----
            # q0, k0 first (psSa/psSb) so pair-0 S^T can start early
            qk_tile(0, on_act=True, bases=[psSa])
            qk_tile(4, on_act=True, bases=[psSb])

            # pair 0: S^T+exp (all ACT; PE-bound anyway) + v tiles (psV) +
            # remaining qk tiles (psX)
            alloc_pT(0)
            rest = [1, 5, 2, 6, 3, 7]        # q1,k1,q2,k2,q3,k3
            for g in range(16):
                if g % 2 == 0:
                    v_tile(g // 2)
                elif g < 13:
                    qk_tile(rest[g // 2], on_act=False, bases=[psSc])
                st_half(0, g, on_dve=False, bufs3=[psSa, psSb])

            # pairs 1..3: PV(t-1) + S^T(t) + exp (ACT/DVE split) + stage_b
            for t in range(1, 4):
                alloc_pT(t)
                pv = t - 1
                for g in range(16):
                    if g == 2 and pv >= 1:
                        pv_evac(pv - 1, 3)
                    if g % 2 == 0:
                        pv_chunk(pv, g // 4, (g // 2) % 2)
                    st_half(t, g, on_dve=(g in EXP_DVE),
                            bufs3=[psSa, psSb, psSc])
                    if g in (6, 10, 14):
                        pv_evac(pv, (g - 6) // 4)
                del pT_tiles[t - 1]

            # PV(3) + early proj partials (ot0/ot1/ot2 in psSa/psSb/psSc)
            proj_sched = {1: (0, 0), 2: (1, 0), 3: (0, 1), 4: (1, 1),
                          5: (0, 2), 7: (1, 2), 9: (2, 0), 11: (2, 1),
                          13: (2, 2)}
            proj_base = {0: psSa, 1: psSb, 2: psSc}
            for g in range(16):
                if g == 2:
                    pv_evac(2, 3)
                ps = proj_sched.get(g)
                if ps is not None:
                    ot, kt = ps
                    proj_chunk(ot, [kt], proj_base[ot],
                               first=(kt == 0), last=False)
                if g % 2 == 0:
                    pv_chunk(3, g // 4, (g // 2) % 2)
                if g in (6, 10, 14):
                    pv_evac(3, (g - 6) // 4)
            pv_evac(3, 3)

            # ---- finish projection + residual ----
            proj_chunk(0, [3], psSa, first=False, last=True)
            proj_finish(0, psSa)
            proj_chunk(1, [3], psSb, first=False, last=True)
            proj_finish(1, psSb)
            proj_chunk(2, [3], psSc, first=False, last=True)
            proj_finish(2, psSc)
            proj_chunk(3, [0, 1, 2, 3], psSa, first=True, last=True)
            proj_finish(3, psSa)

    nc.compile()
    return nc


def _host_prep(x, gn_w, gn_b, qkv_w, qkv_b, proj_w, proj_b):
    xf = np.ascontiguousarray(x.reshape(B, C, N), dtype=np.float32)
    import ml_dtypes
    qkv_wT = np.ascontiguousarray(qkv_w.T).astype(ml_dtypes.bfloat16)
    proj_wT = np.ascontiguousarray(proj_w.T).astype(ml_dtypes.bfloat16)
    proj_be = (proj_b + proj_w @ qkv_b[2 * C:]).astype(np.float32).reshape(C, 1)
    qk_bias = np.ascontiguousarray(qkv_b[:2 * C], dtype=np.float32).reshape(2 * C, 1)
    cid = np.arange(128)
    sel = ((cid[:, None] // GS == np.arange(8)[None, :]) / GS).astype(np.float32)
    expander = np.ascontiguousarray(
        (cid[:, None] // GS == np.arange(8)[None, :]).T.astype(np.float32))
    shared = {
        "qkv_wT": qkv_wT, "proj_wT": proj_wT, "qk_bias": qk_bias,
        "gn_w": np.asarray(gn_w, np.float32).reshape(C, 1),
        "gn_b": np.asarray(gn_b, np.float32).reshape(C, 1),
        "proj_be": proj_be, "sel": sel, "expander": expander,
    }
    return [{**shared, "x": np.ascontiguousarray(xf[i]),
             "x_bf": xf[i].astype(ml_dtypes.bfloat16)} for i in range(B)]


def kernel(x, gn_w, gn_b, qkv_w, qkv_b, proj_w, proj_b):
    from concourse import bass_utils
    in_maps = _host_prep(np.asarray(x), np.asarray(gn_w), np.asarray(gn_b),
                         np.asarray(qkv_w), np.asarray(qkv_b),
                         np.asarray(proj_w), np.asarray(proj_b))
    with_bias = bool(np.any(np.asarray(qkv_b)[:2 * C] != 0.0))
    key = ("nc", with_bias)
    if key not in _cache:
        _cache[key] = _build(with_bias)
    res = bass_utils.run_bass_kernel_spmd(_cache[key], in_maps,
                                          core_ids=list(range(B)), trace=TRACE)
    _cache["last_result"] = res
    out = np.stack([res.results[i]["out"] for i in range(B)])
    return out.reshape(B, C, 32, 32).astype(np.float32)


# revision 20
# speedup vs baseline: 1.2182x; 1.2116x over previous
"""AttentionBlock (GroupNorm32 + 8-head global self-attention + proj + residual)
on 8 TRN2 NeuronCores, data-parallel over batch (B=8 -> 1 image per core).

Per-core layout ([C=512, N=1024] slice, channels on partitions):
  Startup: x shipped twice (bf16 for GN/QKV critical path, f32 late for the
  residual); per-ct DMAs across sync+scalar queues; weights per-kt on gpsimd.
  GroupNorm per-ct as tiles arrive (bn_stats -> per-group selector matmul ->
  expand, affine folded into A,B).  GN psum lives in psSc's upper half; qk
  tiles accumulate nt-outer so those writes are FIFO-gated behind hn3
  (PSUM bank collisions between engines are fatal and the tracker is
  address-level, so bank sharing is managed by emission order).
  Attention: S^T per (m-tile, head-parity) in double-buffered psS halves
  [128,1024]; exp split between ACT (Exp activation) and DVE (Schraudolph
  bit-trick: round(A*s+B) as int16 == bf16 exp approximation, one
  tensor_scalar).  PV per (head, nt) chain accumulates [66,512] with a
  ones-row denominator; evacuation fused with softmax normalization
  (tensor_tensor multiply by DRAM-bounce-broadcast reciprocal); reciprocal
  on repacked [128,2,4] layout.  proj: partial kt accumulation early in
  psS/psV, residual + bias via scalar_tensor_tensor, split output DMA.
"""
import math

import numpy as np

C = 512
NH = 8
D = 64
N = 1024
GROUPS = 32
GS = C // GROUPS  # 16 channels per group
EPS = 1e-5
B = 8
NT = N // 512     # 2 n-tiles of 512
CT = C // 128     # 4 channel tiles
MT = N // 128     # 8 m-tiles (sequence on partitions)

TRACE = False     # test.py flips this for profiling runs

_cache = {}


def _build(with_bias):
    import concourse.bass as bass
    import concourse.bacc as bacc
    import concourse.tile as tile
    import concourse.mybir as mybir

    F32 = mybir.dt.float32
    F32R = mybir.dt.float32r
    BF16 = mybir.dt.bfloat16
    I16 = mybir.dt.int16
    AF = mybir.ActivationFunctionType
    ALU = mybir.AluOpType
    nc = bacc.Bacc("TRN2", target_bir_lowering=False, debug=False,
                   enable_asserts=False, num_devices=1)

    x_d = nc.dram_tensor("x", [C, N], F32, kind="ExternalInput").ap()
    xbf_d = nc.dram_tensor("x_bf", [C, N], BF16, kind="ExternalInput").ap()
    qkv_wT_d = nc.dram_tensor("qkv_wT", [C, 3 * C], BF16, kind="ExternalInput").ap()
    proj_wT_d = nc.dram_tensor("proj_wT", [C, C], BF16, kind="ExternalInput").ap()
    qk_bias_d = nc.dram_tensor("qk_bias", [2 * C, 1], F32, kind="ExternalInput").ap()
    gn_w_d = nc.dram_tensor("gn_w", [C, 1], F32, kind="ExternalInput").ap()
    gn_b_d = nc.dram_tensor("gn_b", [C, 1], F32, kind="ExternalInput").ap()
    proj_be_d = nc.dram_tensor("proj_be", [C, 1], F32, kind="ExternalInput").ap()
    sel_d = nc.dram_tensor("sel", [128, 8], F32R, kind="ExternalInput").ap()
    expander_d = nc.dram_tensor("expander", [8, 128], F32R, kind="ExternalInput").ap()
    rs2_dram = nc.dram_tensor("rs2_scratch", [NH, N], F32, kind="Internal").ap()
    out_d = nc.dram_tensor("out", [C, N], F32, kind="ExternalOutput").ap()

    x_r = x_d.rearrange("(t p) n -> p t n", p=128)
    xbf_r = xbf_d.rearrange("(t p) n -> p t n", p=128)
    qkv_r = qkv_wT_d.rearrange("(t p) o -> p t o", p=128)
    proj_r = proj_wT_d.rearrange("(t p) o -> p t o", p=128)
    out_r = out_d.rearrange("(t p) n -> p t n", p=128)

    scale = float(D) ** -0.5
    # Schraudolph exp-as-bf16: bf16_bits(exp(scale*s)) ~= round(A*s + B)
    SCH_A = (2.0 ** 23) / math.log(2.0) / 65536.0 * scale
    SCH_B = (127.0 * 2 ** 23 - 368000.0) / 65536.0
    # which S^T groups (g = 2*mt + hh) run on DVE instead of ACT, pairs 1-3
    EXP_DVE = {1, 5, 9, 13}

    with tile.TileContext(nc) as tc:
        with tc.tile_pool(name="const", bufs=1) as const, \
             tc.tile_pool(name="big", bufs=1) as big, \
             tc.tile_pool(name="pT_pool", bufs=4) as pT_pool, \
             tc.tile_pool(name="small", bufs=2) as small, \
             tc.tile_pool(name="norm", bufs=4) as norm, \
             tc.tile_pool(name="psSa_p", bufs=1, space="PSUM") as psSa_p, \
             tc.tile_pool(name="psSb_p", bufs=1, space="PSUM") as psSb_p, \
             tc.tile_pool(name="psSc_p", bufs=1, space="PSUM") as psSc_p, \
             tc.tile_pool(name="psVa_p", bufs=1, space="PSUM") as psVa_p, \
             tc.tile_pool(name="psVb_p", bufs=1, space="PSUM") as psVb_p:

            # ---- PSUM: 5 fixed tiles (8 banks). Separate tiles because the
            # dependency tracker serializes at tile granularity; S^T/exp uses
            # a 3-buffer rotation so the engine throughput (not the serial
            # st->exp chain) bounds the cadence.
            psSa = psSa_p.tile([128, 1024], F32)  # S^T rot 0, q0, proj 0/3
            psSb = psSb_p.tile([128, 1024], F32)  # S^T rot 1, k0, proj 1
            psSc = psSc_p.tile([128, 1024], F32)  # S^T rot 2, GN, qk rest, proj 2
            psVa = psVa_p.tile([128, 512], F32)   # v even, PV chains 0,2
            psVb = psVb_p.tile([128, 512], F32)   # v odd, PV chains 1,3

            # ---- constants / weights (gpsimd queue) ----
            sel = const.tile([128, 8], F32R)
            expander = const.tile([8, 128], F32R)
            gn_w = const.tile([128, CT, 1], F32)
            gn_b = const.tile([128, CT, 1], F32)
            proj_be = const.tile([128, CT, 1], F32)
            qkv_wT = const.tile([128, CT, 3 * C], BF16)
            proj_wT = const.tile([128, CT, C], BF16)
            eps_t = const.tile([8, 1], F32)
            qk_bias = const.tile([128, 2 * CT, 1], F32)

            nc.gpsimd.dma_start(out=sel, in_=sel_d)
            nc.gpsimd.dma_start(out=expander, in_=expander_d)
            nc.gpsimd.dma_start(out=gn_w, in_=gn_w_d.rearrange("(t p) o -> p t o", p=128))
            nc.gpsimd.dma_start(out=gn_b, in_=gn_b_d.rearrange("(t p) o -> p t o", p=128))
            nc.gpsimd.dma_start(out=proj_be, in_=proj_be_d.rearrange("(t p) o -> p t o", p=128))
            if with_bias:
                nc.gpsimd.dma_start(out=qk_bias,
                                    in_=qk_bias_d.rearrange("(t p) o -> p t o", p=128))
            for kt in range(CT):
                nc.gpsimd.dma_start(out=qkv_wT[:, kt, :], in_=qkv_r[:, kt, :])
            nc.gpsimd.dma_start(out=proj_wT, in_=proj_r)
            nc.vector.memset(eps_t, EPS)

            # ---- input x (bf16, critical path): 4 per-ct DMAs, 2 queues ----
            xb_sb = big.tile([128, CT, N], BF16)
            for ci in range(CT):
                q = nc.sync if ci % 2 == 0 else nc.scalar
                q.dma_start(out=xb_sb[:, ci, :], in_=xbf_r[:, ci, :])
            # f32 x for the residual, loaded behind the weights (gpsimd)
            x_sb = big.tile([128, CT, N], F32)
            nc.gpsimd.dma_start(out=x_sb, in_=x_r)

            # ---- GroupNorm, per-ct pipeline (psum in psX bank 1) ----
            hn = big.tile([128, CT, N], BF16)
            for ci in range(CT):
                bstats = norm.tile([128, 2, 6], F32, tag="bst")
                xv = xb_sb[:, ci, :].rearrange("p (s n) -> p s n", s=2)
                for s in range(2):
                    nc.vector.bn_stats(out=bstats[:, s, :], in_=xv[:, s, :])
                mv = norm.tile([128, 2], F32, tag="mv")
                nc.vector.bn_aggr(out=mv, in_=bstats)
                # srhs: col0 = mean_c, col1 = var_c + mean_c^2 (= E[x^2])
                srhs = norm.tile([128, 2], F32R, tag="srhs")
                nc.vector.tensor_copy(out=srhs[:, 0:1], in_=mv[:, 0:1])
                nc.vector.tensor_tensor(out=srhs[:, 1:2], in0=mv[:, 0:1],
                                        in1=mv[:, 0:1], op=ALU.mult)
                nc.vector.tensor_tensor(out=srhs[:, 1:2], in0=srhs[:, 1:2],
                                        in1=mv[:, 1:2], op=ALU.add)
                gp = psSc[0:8, 512 + 16 * ci:512 + 16 * ci + 2]
                nc.tensor.matmul(gp, sel[:], srhs[:], start=True, stop=True)
                gms = norm.tile([8, 2], F32, tag="gms")
                nc.vector.tensor_copy(out=gms, in_=gp)
                gvar = norm.tile([8, 1], F32, tag="gvar")
                grp2 = norm.tile([8, 2], F32R, tag="grp2")
                nc.vector.tensor_tensor(out=gvar, in0=gms[:, 0:1], in1=gms[:, 0:1],
                                        op=ALU.mult)
                nc.vector.tensor_tensor(out=gvar, in0=gms[:, 1:2], in1=gvar,
                                        op=ALU.subtract)
                nc.scalar.activation(out=gvar, in_=gvar, func=AF.Sqrt, bias=eps_t,
                                     scale=1.0)
                nc.vector.reciprocal(out=gvar, in_=gvar)
                nc.vector.tensor_copy(out=grp2[:, 0:1], in_=gms[:, 0:1])
                nc.vector.tensor_copy(out=grp2[:, 1:2], in_=gvar)
                ep = psSc[:, 576 + 16 * ci:576 + 16 * ci + 2]
                nc.tensor.matmul(ep, expander[:], grp2[:], start=True, stop=True)
                A = norm.tile([128, 1], F32, tag="A")
                Bb = norm.tile([128, 1], F32, tag="Bb")
                nc.vector.tensor_tensor(out=A, in0=ep[:, 1:2], in1=gn_w[:, ci, :],
                                        op=ALU.mult)
                nc.vector.tensor_tensor(out=Bb, in0=ep[:, 0:1], in1=A, op=ALU.mult)
                nc.vector.tensor_tensor(out=Bb, in0=gn_b[:, ci, :], in1=Bb,
                                        op=ALU.subtract)
                nc.vector.tensor_scalar(out=hn[:, ci, :], in0=xb_sb[:, ci, :],
                                        scalar1=A, scalar2=Bb,
                                        op0=ALU.mult, op1=ALU.add)

            # ---- data tiles for attention ----
            q_sb = big.tile([128, CT, N], BF16)
            k_sb = big.tile([128, CT, N], BF16)
            vT = big.tile([128, MT, NH, D + 2], BF16)
            oT = big.tile([128, CT, N], BF16)
            out_sb = big.tile([128, CT, N], F32)
            nc.vector.memset(vT[:, :, :, D:D + 1], 1.0)
            nc.vector.memset(vT[:, :, :, D + 1:D + 2], 0.0)

            # ---- emission helpers ----
            def qk_tile(i, on_act, bases):
                """QKV tile i (0-3 = q ct, 4-7 = k ct). nt-outer: the nt1
                chain's first MM is FIFO-gated behind nt0's kt3 (needs hn3),
                so psSc's GN half is only written once GN is done."""
                dest = q_sb if i < CT else k_sb
                ci = i % CT
                base = bases[0]
                for nt in range(NT):
                    for kt in range(CT):
                        nc.tensor.matmul(
                            base[:, 512 * nt:512 * (nt + 1)],
                            qkv_wT[:, kt, 128 * i:128 * (i + 1)],
                            hn[:, kt, 512 * nt:512 * (nt + 1)],
                            start=(kt == 0), stop=(kt == CT - 1))
                if with_bias:
                    nc.vector.tensor_scalar(out=dest[:, ci, :],
                                            in0=base[:, 0:1024],
                                            scalar1=qk_bias[:, i, :],
                                            scalar2=None, op0=ALU.add)
                elif on_act:
                    nc.scalar.activation(out=dest[:, ci, :],
                                         in_=base[:, 0:1024], func=AF.Copy)
                else:
                    nc.vector.tensor_copy(out=dest[:, ci, :],
                                          in_=base[:, 0:1024])

            def v_tile(mt):
                """v for n-block mt, evac to vT (head-interleaved)."""
                base = psVa if mt % 2 == 0 else psVb
                for kt in range(CT):
                    nc.tensor.matmul(base[:, 0:512],
                                     hn[:, kt, 128 * mt:128 * (mt + 1)],
                                     qkv_wT[:, kt, 2 * C:3 * C],
                                     start=(kt == 0), stop=(kt == CT - 1))
                nc.vector.tensor_copy(
                    out=vT[:, mt, :, 0:D],
                    in_=base[:, 0:512].rearrange("p (h d) -> p h d", h=NH))

            pT_tiles = {}

            def st_half(t, g, on_dve, bufs3):
                """S^T for head pair t, group g = 2*mt + hh, into a rotating
                psS buffer; exp on ACT or DVE-Schraudolph."""
                mt, hh = g // 2, g % 2
                qp = hh * 64
                half = bufs3[g % len(bufs3)]
                for nt in range(NT):
                    nc.tensor.matmul(
                        half[:, 512 * nt:512 * (nt + 1)],
                        k_sb[qp:qp + 64, t, 128 * mt:128 * (mt + 1)],
                        q_sb[qp:qp + 64, t, 512 * nt:512 * (nt + 1)],
                        start=True, stop=True)
                pTt = pT_tiles[t]
                if on_dve:
                    nc.vector.tensor_scalar(
                        out=pTt.bitcast(I16)[:, hh, mt, :], in0=half,
                        scalar1=SCH_A, scalar2=SCH_B,
                        op0=ALU.mult, op1=ALU.add)
                else:
                    nc.scalar.activation(out=pTt[:, hh, mt, :], in_=half,
                                         func=AF.Exp, scale=scale)

            # PV chain psum slots: chains 0,2 share psVa; 1,3 share psVb
            # (sequential in time: chain c evacs before chain c+2 starts)
            def pv_slot(chain):
                return (psVa if chain % 2 == 0 else psVb)[0:D + 2, 0:512]

            def pv_chunk(t, chain, part):
                """4 MMs of PV chain (0=h_ev/nt0, 1=h_od/nt0, 2=h_ev/nt1,
                3=h_od/nt1), part 0/1 = m-tiles 0-3 / 4-7."""
                hh = chain % 2
                nt = chain // 2
                h = 2 * t + hh
                slot = pv_slot(chain)
                pTt = pT_tiles[t]
                for mt in range(4 * part, 4 * part + 4):
                    nc.tensor.matmul(slot,
                                     vT[:, mt, h, :],
                                     pTt[:, hh, mt, 512 * nt:512 * (nt + 1)],
                                     start=(mt == 0), stop=(mt == MT - 1))
                if part == 1:
                    # denominator row -> SBUF, approx reciprocal (both DVE,
                    # back to back), then DRAM-bounce broadcast
                    rs = small.tile([1, 512], F32, tag="rs", bufs=4,
                                    name=f"rs_{h}_{nt}")
                    rs2 = small.tile([1, 512], F32, tag="rs2", bufs=4,
                                     name=f"rs2_{h}_{nt}")
                    nc.vector.tensor_copy(out=rs, in_=slot[D:D + 1, :])
                    nc.vector.reciprocal_approx_fast(out=rs2, in_=rs)
                    nc.sync.dma_start(out=rs2_dram[h:h + 1, 512 * nt:512 * (nt + 1)],
                                      in_=rs2)
                    bc_load(t, chain)


            bc_tiles = {}

            def bc_load(t, chain):
                """Broadcast 1/denom to 64 partitions via DRAM-bounce DMA."""
                hh, nt = chain % 2, chain // 2
                h = 2 * t + hh
                qp = hh * 64
                key = (t, nt)
                if key not in bc_tiles:
                    bc_tiles[key] = small.tile([128, 512], F32, tag=f"bc{nt}",
                                               bufs=2, name=f"bc_{t}_{nt}")
                bc = bc_tiles[key]
                srcap = rs2_dram[h:h + 1, 512 * nt:512 * (nt + 1)]
                nc.gpsimd.dma_start(out=bc[qp:qp + 64, :],
                                    in_=bass.AP(tensor=srcap.tensor,
                                                offset=srcap.offset,
                                                ap=[[0, 64]] + list(srcap.ap[1:])))

            def pv_evac(t, chain):
                """Fused evacuate+normalize: oT = psum_o * (1/denom)."""
                hh, nt = chain % 2, chain // 2
                qp = hh * 64
                slot = pv_slot(chain)
                bc = bc_tiles[(t, nt)]
                nc.vector.tensor_tensor(
                    out=oT[qp:qp + 64, t, 512 * nt:512 * (nt + 1)],
                    in0=slot[0:D, :], in1=bc[qp:qp + 64, :], op=ALU.mult)

            def proj_chunk(ot, kts, base, first, last):
                for kt in kts:
                    for nt in range(NT):
                        nc.tensor.matmul(
                            base[:, 512 * nt:512 * (nt + 1)],
                            proj_wT[:, kt, 128 * ot:128 * (ot + 1)],
                            oT[:, kt, 512 * nt:512 * (nt + 1)],
                            start=(kt == kts[0] and first),
                            stop=(kt == kts[-1] and last))

            def proj_finish(ot, base):
                nc.vector.scalar_tensor_tensor(
                    out=out_sb[:, ot, :], in0=base[:, 0:1024],
                    scalar=proj_be[:, ot, :], in1=x_sb[:, ot, :],
                    op0=ALU.add, op1=ALU.add)
                q = nc.sync if ot % 2 == 0 else nc.scalar
                q.dma_start(out=out_r[:, ot, :], in_=out_sb[:, ot, :])

            def alloc_pT(t):
                pT_tiles[t] = pT_pool.tile([128, 2, MT, N], BF16, tag="pT", bufs=2,
                                           name=f"pT_{t}")

            # ---- pipeline emission ----
            # q0, k0 first (psSa/psSb) so pair-0 S^T can start early
            qk_tile(0, on_act=True, bases=[psSa])
            qk_tile(4, on_act=True, bases=[psSb])

            # pair 0: S^T+exp (all ACT; PE-bound anyway) + v tiles (psV) +
            # remaining qk tiles (psX)
            alloc_pT(0)
            rest = [1, 5, 2, 6, 3, 7]        # q1,k1,q2,k2,q3,k3
            for g in range(16):
                if g % 2 == 0:
                    v_tile(g // 2)
                elif g < 13:
                    qk_tile(rest[g // 2], on_act=False, bases=[psSc])
                st_half(0, g, on_dve=False, bufs3=[psSa, psSb])

            # pairs 1..3: PV(t-1) + S^T(t) + exp (ACT/DVE split) + stage_b
            for t in range(1, 4):
                alloc_pT(t)
                pv = t - 1
                for g in range(16):
                    if g == 2 and pv >= 1:
                        pv_evac(pv - 1, 3)
                    if g % 2 == 0:
                        pv_chunk(pv, g // 4, (g // 2) % 2)
                    st_half(t, g, on_dve=(g in EXP_DVE),
                            bufs3=[psSa, psSb, psSc])
                    if g in (6, 10, 14):
                        pv_evac(pv, (g - 6) // 4)
                del pT_tiles[t - 1]

            # PV(3) + early proj partials (ot0/ot1/ot2 in psSa/psSb/psSc)
            proj_sched = {1: (0, 0), 2: (1, 0), 3: (0, 1), 4: (1, 1),
                          5: (0, 2), 7: (1, 2), 9: (2, 0), 11: (2, 1),
                          13: (2, 2)}
            proj_base = {0: psSa, 1: psSb, 2: psSc}
            for g in range(16):
                if g == 2:
                    pv_evac(2, 3)
                ps = proj_sched.get(g)
                if ps is not None:
                    ot, kt = ps
                    proj_chunk(ot, [kt], proj_base[ot],
                               first=(kt == 0), last=False)
                if g % 2 == 0:
                    pv_chunk(3, g // 4, (g // 2) % 2)
                if g in (6, 10, 14):
                    pv_evac(3, (g - 6) // 4)
            pv_evac(3, 3)

            # ---- finish projection + residual ----
            proj_chunk(0, [3], psSa, first=False, last=True)
            proj_finish(0, psSa)
            proj_chunk(1, [3], psSb, first=False, last=True)
            proj_finish(1, psSb)
            proj_chunk(2, [3], psSc, first=False, last=True)
            proj_finish(2, psSc)
            proj_chunk(3, [0, 1, 2, 3], psSa, first=True, last=True)
            proj_finish(3, psSa)

    nc.compile()
    return nc


def _host_prep(x, gn_w, gn_b, qkv_w, qkv_b, proj_w, proj_b):
    xf = np.ascontiguousarray(x.reshape(B, C, N), dtype=np.float32)
    import ml_dtypes
    qkv_wT = np.ascontiguousarray(qkv_w.T).astype(ml_dtypes.bfloat16)
    proj_wT = np.ascontiguousarray(proj_w.T).astype(ml_dtypes.bfloat16)
    proj_be = (proj_b + proj_w @ qkv_b[2 * C:]).astype(np.float32).reshape(C, 1)
    qk_bias = np.ascontiguousarray(qkv_b[:2 * C], dtype=np.float32).reshape(2 * C, 1)
    cid = np.arange(128)
    sel = ((cid[:, None] // GS == np.arange(8)[None, :]) / GS).astype(np.float32)
    expander = np.ascontiguousarray(
        (cid[:, None] // GS == np.arange(8)[None, :]).T.astype(np.float32))
    shared = {
        "qkv_wT": qkv_wT, "proj_wT": proj_wT, "qk_bias": qk_bias,
        "gn_w": np.asarray(gn_w, np.float32).reshape(C, 1),
        "gn_b": np.asarray(gn_b, np.float32).reshape(C, 1),
        "proj_be": proj_be, "sel": sel, "expander": expander,
    }
    return [{**shared, "x": np.ascontiguousarray(xf[i]),
             "x_bf": xf[i].astype(ml_dtypes.bfloat16)} for i in range(B)]


def kernel(x, gn_w, gn_b, qkv_w, qkv_b, proj_w, proj_b):
    from concourse import bass_utils
    in_maps = _host_prep(np.asarray(x), np.asarray(gn_w), np.asarray(gn_b),
                         np.asarray(qkv_w), np.asarray(qkv_b),
                         np.asarray(proj_w), np.asarray(proj_b))
    with_bias = bool(np.any(np.asarray(qkv_b)[:2 * C] != 0.0))
    key = ("nc", with_bias)
    if key not in _cache:
        _cache[key] = _build(with_bias)
    res = bass_utils.run_bass_kernel_spmd(_cache[key], in_maps,
                                          core_ids=list(range(B)), trace=TRACE)
    _cache["last_result"] = res
    out = np.stack([res.results[i]["out"] for i in range(B)])
    return out.reshape(B, C, 32, 32).astype(np.float32)


# revision 25
# speedup vs baseline: 1.2570x; 1.0318x over previous
"""AttentionBlock (GroupNorm32 + 8-head global self-attention + proj + residual)
on 8 TRN2 NeuronCores, data-parallel over batch (B=8 -> 1 image per core).

Per-core layout ([C=512, N=1024] slice, channels on partitions):
  Startup: x shipped twice (bf16 for GN/QKV critical path, f32 late for the
  residual); per-ct DMAs across sync+scalar queues; weights per-kt on gpsimd.
  GroupNorm per-ct as tiles arrive (bn_stats -> per-group selector matmul ->
  expand, affine folded into A,B).  GN psum lives in psSc's upper half; qk
  tiles accumulate nt-outer so those writes are FIFO-gated behind hn3
  (PSUM bank collisions between engines are fatal and the tracker is
  address-level, so bank sharing is managed by emission order).
  Attention: S^T per (m-tile, head-parity) in double-buffered psS halves
  [128,1024]; exp split between ACT (Exp activation) and DVE (Schraudolph
  bit-trick: round(A*s+B) as int16 == bf16 exp approximation, one
  tensor_scalar).  PV per (head, nt) chain accumulates [66,512] with a
  ones-row denominator; evacuation fused with softmax normalization
  (tensor_tensor multiply by DRAM-bounce-broadcast reciprocal); reciprocal
  on repacked [128,2,4] layout.  proj: partial kt accumulation early in
  psS/psV, residual + bias via scalar_tensor_tensor, split output DMA.
"""
import math

import numpy as np

C = 512
NH = 8
D = 64
N = 1024
GROUPS = 32
GS = C // GROUPS  # 16 channels per group
EPS = 1e-5
B = 8
NT = N // 512     # 2 n-tiles of 512
CT = C // 128     # 4 channel tiles
MT = N // 128     # 8 m-tiles (sequence on partitions)

TRACE = False     # test.py flips this for profiling runs

_cache = {}


def _build(with_bias):
    import concourse.bass as bass
    import concourse.bacc as bacc
    import concourse.tile as tile
    import concourse.mybir as mybir

    F32 = mybir.dt.float32
    F32R = mybir.dt.float32r
    BF16 = mybir.dt.bfloat16
    I16 = mybir.dt.int16
    AF = mybir.ActivationFunctionType
    ALU = mybir.AluOpType
    nc = bacc.Bacc("TRN2", target_bir_lowering=False, debug=False,
                   enable_asserts=False, num_devices=1)

    x_d = nc.dram_tensor("x", [C, N], F32, kind="ExternalInput").ap()
    xbf_d = nc.dram_tensor("x_bf", [C, N], BF16, kind="ExternalInput").ap()
    qkv_wT_d = nc.dram_tensor("qkv_wT", [C, 3 * C], BF16, kind="ExternalInput").ap()
    proj_wT_d = nc.dram_tensor("proj_wT", [C, C], BF16, kind="ExternalInput").ap()
    qk_bias_d = nc.dram_tensor("qk_bias", [2 * C, 1], F32, kind="ExternalInput").ap()
    gn_w_d = nc.dram_tensor("gn_w", [C, 1], F32, kind="ExternalInput").ap()
    gn_b_d = nc.dram_tensor("gn_b", [C, 1], F32, kind="ExternalInput").ap()
    proj_be_d = nc.dram_tensor("proj_be", [C, 1], F32, kind="ExternalInput").ap()
    sel_d = nc.dram_tensor("sel", [128, 8], F32R, kind="ExternalInput").ap()
    expander_d = nc.dram_tensor("expander", [8, 128], F32R, kind="ExternalInput").ap()
    rs2_dram = nc.dram_tensor("rs2_scratch", [NH, N], F32, kind="Internal").ap()
    out_d = nc.dram_tensor("out", [C, N], F32, kind="ExternalOutput").ap()

    x_r = x_d.rearrange("(t p) n -> p t n", p=128)
    xbf_r = xbf_d.rearrange("(t p) n -> p t n", p=128)
    qkv_r = qkv_wT_d.rearrange("(t p) o -> p t o", p=128)
    proj_r = proj_wT_d.rearrange("(t p) o -> p t o", p=128)
    out_r = out_d.rearrange("(t p) n -> p t n", p=128)

    scale = float(D) ** -0.5
    # Schraudolph exp-as-bf16: bf16_bits(exp(scale*s)) ~= round(A*s + B)
    SCH_A = (2.0 ** 23) / math.log(2.0) / 65536.0 * scale
    SCH_B = (127.0 * 2 ** 23 - 368000.0) / 65536.0
    # which S^T groups (g = 2*mt + hh) run on DVE instead of ACT, pairs 1-3
    EXP_DVE = {1, 5, 9, 13}

    with tile.TileContext(nc) as tc:
        with tc.tile_pool(name="const", bufs=1) as const, \
             tc.tile_pool(name="big", bufs=1) as big, \
             tc.tile_pool(name="pT_pool", bufs=4) as pT_pool, \
             tc.tile_pool(name="small", bufs=2) as small, \
             tc.tile_pool(name="norm", bufs=4) as norm, \
             tc.tile_pool(name="psSa_p", bufs=1, space="PSUM") as psSa_p, \
             tc.tile_pool(name="psSb_p", bufs=1, space="PSUM") as psSb_p, \
             tc.tile_pool(name="psSc_p", bufs=1, space="PSUM") as psSc_p, \
             tc.tile_pool(name="psVa_p", bufs=1, space="PSUM") as psVa_p, \
             tc.tile_pool(name="psVb_p", bufs=1, space="PSUM") as psVb_p:

            # ---- PSUM: 5 fixed tiles (8 banks). Separate tiles because the
            # dependency tracker serializes at tile granularity; S^T/exp uses
            # a 3-buffer rotation so the engine throughput (not the serial
            # st->exp chain) bounds the cadence.
            psSa = psSa_p.tile([128, 1024], F32)  # S^T rot 0, q0, proj 0/3
            psSb = psSb_p.tile([128, 1024], F32)  # S^T rot 1, k0, proj 1
            psSc = psSc_p.tile([128, 1024], F32)  # S^T rot 2, GN, qk rest, proj 2
            psVa = psVa_p.tile([128, 512], F32)   # v even, PV chains 0,2
            psVb = psVb_p.tile([128, 512], F32)   # v odd, PV chains 1,3

            # ---- constants / weights (gpsimd queue) ----
            sel = const.tile([128, 8], F32R)
            expander = const.tile([8, 128], F32R)
            gn_w = const.tile([128, CT, 1], F32)
            gn_b = const.tile([128, CT, 1], F32)
            proj_be = const.tile([128, CT, 1], F32)
            qkv_wT = const.tile([128, CT, 3 * C], BF16)
            proj_wT = const.tile([128, CT, C], BF16)
            eps_t = const.tile([8, 1], F32)
            ones_f = const.tile([1, 64], F32)
            ones_r = const.tile([1, 64], F32R)
            qk_bias = const.tile([128, 2 * CT, 1], F32)

            nc.gpsimd.dma_start(out=sel, in_=sel_d)
            nc.gpsimd.dma_start(out=expander, in_=expander_d)
            nc.gpsimd.dma_start(out=gn_w, in_=gn_w_d.rearrange("(t p) o -> p t o", p=128))
            nc.gpsimd.dma_start(out=gn_b, in_=gn_b_d.rearrange("(t p) o -> p t o", p=128))
            nc.gpsimd.dma_start(out=proj_be, in_=proj_be_d.rearrange("(t p) o -> p t o", p=128))
            if with_bias:
                nc.gpsimd.dma_start(out=qk_bias,
                                    in_=qk_bias_d.rearrange("(t p) o -> p t o", p=128))
            for kt in range(CT):
                nc.gpsimd.dma_start(out=qkv_wT[:, kt, :], in_=qkv_r[:, kt, :])
            nc.gpsimd.dma_start(out=proj_wT, in_=proj_r)
            nc.vector.memset(eps_t, EPS)
            nc.vector.memset(ones_f, 1.0)
            nc.vector.tensor_copy(out=ones_r, in_=ones_f)

            # ---- input x (bf16, critical path): 4 per-ct DMAs, 2 queues ----
            xb_sb = big.tile([128, CT, N], BF16)
            for ci in range(CT):
                q = nc.sync if ci % 2 == 0 else nc.scalar
                q.dma_start(out=xb_sb[:, ci, :], in_=xbf_r[:, ci, :])
            # f32 x for the residual, loaded behind the weights (gpsimd)
            x_sb = big.tile([128, CT, N], F32)
            nc.gpsimd.dma_start(out=x_sb, in_=x_r)

            # ---- GroupNorm, per-ct pipeline (psum in psX bank 1) ----
            hn = big.tile([128, CT, N], BF16)
            for ci in range(CT):
                bstats = norm.tile([128, 2, 6], F32, tag="bst")
                xv = xb_sb[:, ci, :].rearrange("p (s n) -> p s n", s=2)
                for s in range(2):
                    nc.vector.bn_stats(out=bstats[:, s, :], in_=xv[:, s, :])
                mv = norm.tile([128, 2], F32, tag="mv")
                nc.vector.bn_aggr(out=mv, in_=bstats)
                # srhs: col0 = mean_c, col1 = var_c + mean_c^2 (= E[x^2])
                srhs = norm.tile([128, 2], F32R, tag="srhs")
                nc.vector.tensor_copy(out=srhs[:, 0:1], in_=mv[:, 0:1])
                nc.vector.tensor_tensor(out=srhs[:, 1:2], in0=mv[:, 0:1],
                                        in1=mv[:, 0:1], op=ALU.mult)
                nc.vector.tensor_tensor(out=srhs[:, 1:2], in0=srhs[:, 1:2],
                                        in1=mv[:, 1:2], op=ALU.add)
                gp = psSc[0:8, 512 + 16 * ci:512 + 16 * ci + 2]
                nc.tensor.matmul(gp, sel[:], srhs[:], start=True, stop=True)
                gms = norm.tile([8, 2], F32, tag="gms")
                nc.vector.tensor_copy(out=gms, in_=gp)
                gvar = norm.tile([8, 1], F32, tag="gvar")
                grp2 = norm.tile([8, 2], F32R, tag="grp2")
                nc.vector.tensor_tensor(out=gvar, in0=gms[:, 0:1], in1=gms[:, 0:1],
                                        op=ALU.mult)
                nc.vector.tensor_tensor(out=gvar, in0=gms[:, 1:2], in1=gvar,
                                        op=ALU.subtract)
                nc.scalar.activation(out=gvar, in_=gvar, func=AF.Sqrt, bias=eps_t,
                                     scale=1.0)
                nc.vector.reciprocal(out=gvar, in_=gvar)
                nc.vector.tensor_copy(out=grp2[:, 0:1], in_=gms[:, 0:1])
                nc.vector.tensor_copy(out=grp2[:, 1:2], in_=gvar)
                ep = psSc[:, 576 + 16 * ci:576 + 16 * ci + 2]
                nc.tensor.matmul(ep, expander[:], grp2[:], start=True, stop=True)
                A = norm.tile([128, 1], F32, tag="A")
                Bb = norm.tile([128, 1], F32, tag="Bb")
                nc.vector.tensor_tensor(out=A, in0=ep[:, 1:2], in1=gn_w[:, ci, :],
                                        op=ALU.mult)
                nc.vector.tensor_tensor(out=Bb, in0=ep[:, 0:1], in1=A, op=ALU.mult)
                nc.vector.tensor_tensor(out=Bb, in0=gn_b[:, ci, :], in1=Bb,
                                        op=ALU.subtract)
                nc.vector.tensor_scalar(out=hn[:, ci, :], in0=xb_sb[:, ci, :],
                                        scalar1=A, scalar2=Bb,
                                        op0=ALU.mult, op1=ALU.add)

            # ---- data tiles for attention ----
            q_sb = big.tile([128, CT, N], BF16)
            k_sb = big.tile([128, CT, N], BF16)
            vT = big.tile([128, MT, NH, D + 2], BF16)
            oT = big.tile([128, CT, N], BF16)
            out_sb = big.tile([128, CT, N], F32)
            nc.vector.memset(vT[:, :, :, D:D + 1], 1.0)
            nc.vector.memset(vT[:, :, :, D + 1:D + 2], 0.0)

            # ---- emission helpers ----
            def qk_tile(i, on_act, bases):
                """QKV tile i (0-3 = q ct, 4-7 = k ct). nt-outer: the nt1
                chain's first MM is FIFO-gated behind nt0's kt3 (needs hn3),
                so psSc's GN half is only written once GN is done."""
                dest = q_sb if i < CT else k_sb
                ci = i % CT
                base = bases[0]
                for nt in range(NT):
                    for kt in range(CT):
                        nc.tensor.matmul(
                            base[:, 512 * nt:512 * (nt + 1)],
                            qkv_wT[:, kt, 128 * i:128 * (i + 1)],
                            hn[:, kt, 512 * nt:512 * (nt + 1)],
                            start=(kt == 0), stop=(kt == CT - 1))
                if with_bias:
                    nc.vector.tensor_scalar(out=dest[:, ci, :],
                                            in0=base[:, 0:1024],
                                            scalar1=qk_bias[:, i, :],
                                            scalar2=None, op0=ALU.add)
                elif on_act:
                    nc.scalar.activation(out=dest[:, ci, :],
                                         in_=base[:, 0:1024], func=AF.Copy)
                else:
                    nc.vector.tensor_copy(out=dest[:, ci, :],
                                          in_=base[:, 0:1024])

            def v_tile(mt):
                """v for n-block mt, evac to vT (head-interleaved)."""
                base = psVa if mt % 2 == 0 else psVb
                for kt in range(CT):
                    nc.tensor.matmul(base[:, 0:512],
                                     hn[:, kt, 128 * mt:128 * (mt + 1)],
                                     qkv_wT[:, kt, 2 * C:3 * C],
                                     start=(kt == 0), stop=(kt == CT - 1))
                nc.vector.tensor_copy(
                    out=vT[:, mt, :, 0:D],
                    in_=base[:, 0:512].rearrange("p (h d) -> p h d", h=NH))

            pT_tiles = {}

            def st_half(t, g, on_dve, bufs3):
                """S^T for head pair t, group g = 2*mt + hh, into a rotating
                psS buffer; exp on ACT or DVE-Schraudolph."""
                mt, hh = g // 2, g % 2
                qp = hh * 64
                half = bufs3[g % len(bufs3)]
                for nt in range(NT):
                    nc.tensor.matmul(
                        half[:, 512 * nt:512 * (nt + 1)],
                        k_sb[qp:qp + 64, t, 128 * mt:128 * (mt + 1)],
                        q_sb[qp:qp + 64, t, 512 * nt:512 * (nt + 1)],
                        start=True, stop=True)
                pTt = pT_tiles[t]
                if on_dve:
                    nc.vector.tensor_scalar(
                        out=pTt.bitcast(I16)[:, hh, mt, :], in0=half,
                        scalar1=SCH_A, scalar2=SCH_B,
                        op0=ALU.mult, op1=ALU.add)
                else:
                    nc.scalar.activation(out=pTt[:, hh, mt, :], in_=half,
                                         func=AF.Exp, scale=scale)

            # PV chain psum slots: chains 0,2 share psVa; 1,3 share psVb
            # (sequential in time: chain c evacs before chain c+2 starts)
            def pv_slot(chain):
                return (psVa if chain % 2 == 0 else psVb)[0:D + 2, 0:512]

            def pv_chunk(t, chain, part):
                """4 MMs of PV chain (0=h_ev/nt0, 1=h_od/nt0, 2=h_ev/nt1,
                3=h_od/nt1), part 0/1 = m-tiles 0-3 / 4-7."""
                hh = chain % 2
                nt = chain // 2
                h = 2 * t + hh
                slot = pv_slot(chain)
                pTt = pT_tiles[t]
                for mt in range(4 * part, 4 * part + 4):
                    nc.tensor.matmul(slot,
                                     vT[:, mt, h, :],
                                     pTt[:, hh, mt, 512 * nt:512 * (nt + 1)],
                                     start=(mt == 0), stop=(mt == MT - 1))
                if part == 1:
                    # denominator row -> SBUF, approx reciprocal (both DVE,
                    # back to back)
                    rs = small.tile([1, 512], F32, tag="rs", bufs=4,
                                    name=f"rs_{h}_{nt}")
                    rs2 = small.tile([1, 512], F32, tag="rs2", bufs=4,
                                     name=f"rs2_{h}_{nt}")
                    nc.vector.tensor_copy(out=rs, in_=slot[D:D + 1, :])
                    nc.vector.reciprocal_approx_fast(out=rs2, in_=rs)
                    if (t, chain) in PE_BCAST:
                        rs2r = small.tile([1, 512], F32R, tag="rs2r", bufs=2,
                                          name=f"rs2r_{h}_{nt}")
                        nc.vector.tensor_copy(out=rs2r, in_=rs2)
                        rs2_sb[(t, chain)] = rs2r
                    else:
                        nc.sync.dma_start(
                            out=rs2_dram[h:h + 1, 512 * nt:512 * (nt + 1)],
                            in_=rs2)
                        bc_load(t, chain)


            bc_tiles = {}

            def bc_load(t, chain):
                """Broadcast 1/denom to 64 partitions via DRAM-bounce DMA."""
                hh, nt = chain % 2, chain // 2
                h = 2 * t + hh
                qp = hh * 64
                key = (t, nt)
                if key not in bc_tiles:
                    bc_tiles[key] = small.tile([128, 512], F32, tag=f"bc{nt}",
                                               bufs=2, name=f"bc_{t}_{nt}")
                bc = bc_tiles[key]
                srcap = rs2_dram[h:h + 1, 512 * nt:512 * (nt + 1)]
                nc.gpsimd.dma_start(out=bc[qp:qp + 64, :],
                                    in_=bass.AP(tensor=srcap.tensor,
                                                offset=srcap.offset,
                                                ap=[[0, 64]] + list(srcap.ap[1:])))

            rs2_sb = {}
            PE_BCAST = set()

            def pv_evac_pe(t, chain):
                """On-chip broadcast: ones^T @ rs2 into spare PSUM rows of the
                OTHER psV tile, copy to SBUF, then the fused normalize evac.
                Avoids the ~6us DRAM-bounce latency on the critical tail."""
                hh, nt = chain % 2, chain // 2
                qp = hh * 64
                other = psVb if chain % 2 == 0 else psVa
                bcp = other[64:128, 0:512]
                nc.tensor.matmul(bcp, ones_r,
                                 rs2_sb[(t, chain)],
                                 start=True, stop=True)
                bcs = small.tile([64, 512], F32, tag="bcs", bufs=2,
                                 name=f"bcs_{t}_{chain}")
                nc.vector.tensor_copy(out=bcs, in_=bcp)
                slot = pv_slot(chain)
                nc.vector.tensor_tensor(
                    out=oT[qp:qp + 64, t, 512 * nt:512 * (nt + 1)],
                    in0=slot[0:D, :], in1=bcs, op=ALU.mult)

            def pv_evac(t, chain):
                """Fused evacuate+normalize: oT = psum_o * (1/denom)."""
                hh, nt = chain % 2, chain // 2
                qp = hh * 64
                slot = pv_slot(chain)
                bc = bc_tiles[(t, nt)]
                nc.vector.tensor_tensor(
                    out=oT[qp:qp + 64, t, 512 * nt:512 * (nt + 1)],
                    in0=slot[0:D, :], in1=bc[qp:qp + 64, :], op=ALU.mult)

            def proj_chunk(ot, kts, base, first, last):
                for kt in kts:
                    for nt in range(NT):
                        nc.tensor.matmul(
                            base[:, 512 * nt:512 * (nt + 1)],
                            proj_wT[:, kt, 128 * ot:128 * (ot + 1)],
                            oT[:, kt, 512 * nt:512 * (nt + 1)],
                            start=(kt == kts[0] and first),
                            stop=(kt == kts[-1] and last))

            def proj_finish(ot, base):
                nc.vector.scalar_tensor_tensor(
                    out=out_sb[:, ot, :], in0=base[:, 0:1024],
                    scalar=proj_be[:, ot, :], in1=x_sb[:, ot, :],
                    op0=ALU.add, op1=ALU.add)
                q = nc.sync if ot % 2 == 0 else nc.scalar
                q.dma_start(out=out_r[:, ot, :], in_=out_sb[:, ot, :])

            def alloc_pT(t):
                pT_tiles[t] = pT_pool.tile([128, 2, MT, N], BF16, tag="pT", bufs=2,
                                           name=f"pT_{t}")

            # ---- pipeline emission ----
            # q0, k0 first (psSa/psSb) so pair-0 S^T can start early
            qk_tile(0, on_act=True, bases=[psSa])
            qk_tile(4, on_act=True, bases=[psSb])

            # pair 0: S^T+exp (all ACT; PE-bound anyway) + v tiles (psV) +
            # remaining qk tiles (psX)
            alloc_pT(0)
            rest = [1, 5, 2, 6, 3, 7]        # q1,k1,q2,k2,q3,k3
            for g in range(16):
                if g % 2 == 0:
                    v_tile(g // 2)
                elif g < 13:
                    qk_tile(rest[g // 2], on_act=False, bases=[psSc])
                st_half(0, g, on_dve=False, bufs3=[psSa, psSb])

            # pairs 1..3: PV(t-1) + S^T(t) + exp (ACT/DVE split) + stage_b
            for t in range(1, 4):
                alloc_pT(t)
                pv = t - 1
                chunk_sched = {0: (0, 0), 2: (0, 1), 4: (1, 0), 6: (1, 1),
                               9: (2, 0), 11: (2, 1), 13: (3, 0), 15: (3, 1)}
                for g in range(16):
                    if g == 2 and pv >= 1:
                        pv_evac(pv - 1, 3)
                    cs = chunk_sched.get(g)
                    if cs is not None:
                        pv_chunk(pv, cs[0], cs[1])
                    st_half(t, g, on_dve=(g in EXP_DVE),
                            bufs3=[psSa, psSb, psSc])
                    if g == 8:
                        pv_evac(pv, 0)
                    elif g == 12:
                        pv_evac(pv, 1)
                    elif g == 15:
                        pv_evac(pv, 2)
                del pT_tiles[t - 1]

            # PV(3) + early proj partials (ot0/ot1/ot2 in psSa/psSb/psSc)
            proj_sched = {1: (0, 0), 2: (1, 0), 3: (0, 1), 4: (1, 1),
                          5: (0, 2), 7: (1, 2), 10: (2, 0), 12: (2, 1),
                          14: (2, 2)}
            proj_base = {0: psSa, 1: psSb, 2: psSc}
            chunk_sched = {0: (0, 0), 2: (0, 1), 4: (1, 0), 6: (1, 1),
                           9: (2, 0), 11: (2, 1), 13: (3, 0), 15: (3, 1)}
            for g in range(16):
                if g == 2:
                    pv_evac(2, 3)
                ps = proj_sched.get(g)
                if ps is not None:
                    ot, kt = ps
                    proj_chunk(ot, [kt], proj_base[ot],
                               first=(kt == 0), last=False)
                cs = chunk_sched.get(g)
                if cs is not None:
                    pv_chunk(3, cs[0], cs[1])
                if g == 8:
                    pv_evac(3, 0)
                elif g == 12:
                    pv_evac(3, 1)
                elif g == 15:
                    pv_evac(3, 2)
            pv_evac(3, 3)

            # ---- finish projection + residual ----
            proj_chunk(0, [3], psSa, first=False, last=True)
            proj_finish(0, psSa)
            proj_chunk(1, [3], psSb, first=False, last=True)
            proj_finish(1, psSb)
            proj_chunk(2, [3], psSc, first=False, last=True)
            proj_finish(2, psSc)
            proj_chunk(3, [0, 1, 2, 3], psSa, first=True, last=True)
            proj_finish(3, psSa)

    nc.compile()
    return nc


def _host_prep(x, gn_w, gn_b, qkv_w, qkv_b, proj_w, proj_b):
    xf = np.ascontiguousarray(x.reshape(B, C, N), dtype=np.float32)
    import ml_dtypes
    qkv_wT = np.ascontiguousarray(qkv_w.T).astype(ml_dtypes.bfloat16)
    proj_wT = np.ascontiguousarray(proj_w.T).astype(ml_dtypes.bfloat16)
    proj_be = (proj_b + proj_w @ qkv_b[2 * C:]).astype(np.float32).reshape(C, 1)
    qk_bias = np.ascontiguousarray(qkv_b[:2 * C], dtype=np.float32).reshape(2 * C, 1)
    cid = np.arange(128)
    sel = ((cid[:, None] // GS == np.arange(8)[None, :]) / GS).astype(np.float32)
    expander = np.ascontiguousarray(
        (cid[:, None] // GS == np.arange(8)[None, :]).T.astype(np.float32))
    shared = {
        "qkv_wT": qkv_wT, "proj_wT": proj_wT, "qk_bias": qk_bias,
        "gn_w": np.asarray(gn_w, np.float32).reshape(C, 1),
        "gn_b": np.asarray(gn_b, np.float32).reshape(C, 1),
        "proj_be": proj_be, "sel": sel, "expander": expander,
    }
    return [{**shared, "x": np.ascontiguousarray(xf[i]),
             "x_bf": xf[i].astype(ml_dtypes.bfloat16)} for i in range(B)]


def kernel(x, gn_w, gn_b, qkv_w, qkv_b, proj_w, proj_b):
    from concourse import bass_utils
    in_maps = _host_prep(np.asarray(x), np.asarray(gn_w), np.asarray(gn_b),
                         np.asarray(qkv_w), np.asarray(qkv_b),
                         np.asarray(proj_w), np.asarray(proj_b))
    with_bias = bool(np.any(np.asarray(qkv_b)[:2 * C] != 0.0))
    key = ("nc", with_bias)
    if key not in _cache:
        _cache[key] = _build(with_bias)
    res = bass_utils.run_bass_kernel_spmd(_cache[key], in_maps,
                                          core_ids=list(range(B)), trace=TRACE)
    _cache["last_result"] = res
    out = np.stack([res.results[i]["out"] for i in range(B)])
    return out.reshape(B, C, 32, 32).astype(np.float32)
